# revision 7
# baseline (speedup 1.0000x reference)
"""Trainium2 Bass kernel v5 for the 19-class mean-IoU (DiceLoss) problem.

Full-input contract: kernel(input, target) takes the FULL unsharded inputs
(input [4,19,512,1024] f32, target [4,512,1024] int), returns the scalar
f32 mean-IoU.  The 2,097,152 pixels are sharded across 8 NeuronCores
(data-parallel over the flattened pixel axis); each core computes
per-class partial counts for its 262,144 pixels; the count vectors are
summed on host and divided there (57 floats).

v5 design (probe-driven):
  - X is loaded HBM->SBUF by the gpsimd software-DGE, which casts
    f32 -> fp16 in flight (bit-exact RNE, verified) -- the cast costs no
    compute and halves SBUF.  All the heavy DVE math then runs on 2-byte
    operands, where TensorTensor ops run in 2x mode (~0.55 ns/elem vs
    1.2 f32).
  - per block [128 x (19 x f)] fp16, class-major:
      DVE: tree max over classes -> M; one whole-tile broadcast
           TT is_equal (X vs M) -> EQF masks; 19 fused STT
           (t==c)*EQF_c + accum -> intersection counts.
      ACT: 19 activation-Copy+accum over EQF_c -> pred counts.
  - label counts: host bincount of the uint8 target (input-only work).
  - final: per-block accumulators reduced on DVE, ones-matmul on the PE
    collapses partitions, one 38-float DMA out per core.
fp16 quantization changes argmax ties (~0.2% of pixels); the exact
quantized metric is simulated on host in test.py (rel err ~1e-3, well
inside the 2e-2 gate).
"""

import numpy as np

import concourse.bass as bass
import concourse.mybir as mybir
from concourse import bass_utils
from concourse.tile import TileContext
from concourse.tile_rust import add_dep_helper

C = 19          # classes
P = 128         # SBUF partitions
NCORES = 8
BLOCKS = (256, 256, 512, 1024)   # per-block pixels/partition (sum 2048)
OUTN = 2 * C    # pred counts + inter counts

_Alu = mybir.AluOpType
_Ax = mybir.AxisListType
_dt = mybir.dt
_Act = mybir.ActivationFunctionType


def _body_v5(tc, x, t, out, n, fs):
    """x: DRAM [C, n] f32, t: DRAM [n] u8, out: DRAM [2C] f32.
    fs: per-block free sizes (pixels per partition), summing to n // P."""
    nc = tc.nc
    nb = len(fs)
    assert sum(fs) == n // P
    fp16 = _dt.float16
    with tc.tile_pool(name="xp", bufs=1) as xp, \
         tc.tile_pool(name="eqp", bufs=1) as eqp, \
         tc.tile_pool(name="mp", bufs=1) as mp, \
         tc.tile_pool(name="accp", bufs=1) as accp, \
         tc.tile_pool(name="psp", bufs=1, space="PSUM") as psp:
        pacc = accp.tile([P, nb * C], _dt.float32, tag="pacc")
        iacc = accp.tile([P, nb * C], _dt.float32, tag="iacc")
        ones = accp.tile([P, 1], _dt.float32, tag="ones")
        nc.vector.memset(ones[:, :], 1.0)

        hooks = []
        # target DMA: per-block tiles (single writer each)
        t8s = []
        off = 0
        for b, f in enumerate(fs):
            T8b = accp.tile([P, f], _dt.uint8, tag=f"T8{b}")
            t8d = nc.sync.dma_start(
                T8b[:, :].rearrange("p (a f) -> p a f", a=2),
                t[P * off:P * (off + f)].rearrange(
                    "(p a f) -> p a f", p=P, a=2),
            )
            hooks.append(t8d)
            t8s.append(T8b)
            off += f
        xs = []
        xd_last = None
        off = 0
        for b, f in enumerate(fs):
            X = xp.tile([P, C * f], fp16, tag=f"X{b}")
            xs.append(X)
            xd = nc.gpsimd.dma_start(
                X[:, :].rearrange("p (c f) -> p c f", c=C),
                x[:, P * off:P * (off + f)].rearrange(
                    "c (p f) -> p c f", p=P),
            )
            hooks.append(xd)
            xd_last = xd
            off += f

        small_dve = []
        last_act = None
        for b, f in enumerate(fs):
            X = xs[b]
            # target cast u8 -> fp16 (values 0..18 exact)
            Tf = mp.tile([P, f], fp16, tag=f"Tf{b}")
            cast = nc.vector.tensor_copy(Tf[:, :], t8s[b][:, :])
            small_dve.append(cast)

            # tree max over 19 classes: 8f-halving, fold 16..18 into the
            # low lanes, then 4f/2f/f halvings.
            # (first op carries this block's X-DMA wait)
            MT = mp.tile([P, 8 * f], fp16, tag="MT")
            M = mp.tile([P, f], fp16, tag="M")
            nc.vector.tensor_tensor(
                MT[:, :], X[:, 0:8 * f], X[:, 8 * f:16 * f], _Alu.max)
            nc.vector.tensor_tensor(
                MT[:, 0:3 * f], MT[:, 0:3 * f], X[:, 16 * f:19 * f], _Alu.max)
            nc.vector.tensor_tensor(
                MT[:, 0:4 * f], MT[:, 0:4 * f], MT[:, 4 * f:8 * f], _Alu.max)
            nc.vector.tensor_tensor(
                MT[:, 0:2 * f], MT[:, 0:2 * f], MT[:, 2 * f:4 * f], _Alu.max)
            nc.vector.tensor_tensor(
                M[:, :], MT[:, 0:f], MT[:, f:2 * f], _Alu.max)

            # EQF whole-tile: (X == M broadcast over classes), one 2x op
            EQF = eqp.tile([P, C * f], fp16, tag=f"EQF{b}")
            eqf = nc.vector.tensor_tensor(
                EQF[:, :].rearrange("p (c f) -> p c f", c=C),
                X[:, :].rearrange("p (c f) -> p c f", c=C),
                M[:, :].rearrange("p (c f) -> p c f", c=1)
                .to_broadcast([P, C, f]),
                _Alu.is_equal)

            # ACT: pred counts = per-class Copy+accum over EQF.
            # Accum ops carry an implicit self-engine wait, so a non-accum
            # entry shim absorbs the cross-engine DVE wait first.
            ASH = mp.tile([P, 1], fp16, tag="ASH")
            ash = nc.scalar.activation(ASH[:, :], EQF[:, 0:1], _Act.Copy)
            if last_act is not None:
                add_dep_helper(ash.ins, last_act.ins, sync=False,
                               reason="chain ACT")
            last_act = ash
            AJ = mp.tile([P, f], fp16, tag="AJ")
            for c in range(C):
                a = nc.scalar.activation(
                    AJ[:, :], EQF[:, c * f:(c + 1) * f], _Act.Copy,
                    accum_out=pacc[:, b * C + c:b * C + c + 1])
                add_dep_helper(a.ins, ash.ins, sync=False,
                               reason="after shim")
                last_act = a

            # DVE: intersection counts, fused (t==c)*EQF_c + accum
            EQJ = mp.tile([P, f], fp16, tag="EQJ")
            for c in range(C):
                nc.vector.scalar_tensor_tensor(
                    out=EQJ[:, :], in0=Tf[:, :], scalar=float(c),
                    in1=EQF[:, c * f:(c + 1) * f],
                    op0=_Alu.is_equal, op1=_Alu.mult,
                    accum_out=iacc[:, b * C + c:b * C + c + 1])

        CNT = accp.tile([P, OUTN], _dt.float32, tag="CNT")
        # pacc is ACT-written: this reduce carries one ACT wait
        nc.vector.tensor_reduce(
            CNT[:, 0:C],
            pacc[:, :].rearrange("p (b c) -> p c b", c=C),
            axis=_Ax.X, op=_Alu.add,
        )
        nc.vector.tensor_reduce(
            CNT[:, C:OUTN],
            iacc[:, :].rearrange("p (b c) -> p c b", c=C),
            axis=_Ax.X, op=_Alu.add,
        )
        PS = psp.tile([1, OUTN], _dt.float32, tag="PS")
        mm = nc.tensor.matmul(
            PS[:, :], ones[:, :], CNT[:, :], start=True, stop=True
        )
        OUT = accp.tile([1, OUTN], _dt.float32, tag="OUT")
        oc = nc.vector.tensor_copy(OUT[:, :], PS[:, :])
        for s in small_dve:
            add_dep_helper(oc.ins, s.ins, sync=False,
                           reason="OUT copy last on DVE")
        od = nc.sync.dma_start(out.rearrange("(o k) -> o k", o=1), OUT[:, :])

        # Pool-engine terminal (covers the SW-DGE DMACopy instruction ticks)
        GSCR = accp.tile([1, 1], _dt.float32, tag="GSCR")
        gfin = nc.gpsimd.memset(GSCR[:, :], 0.0)
        add_dep_helper(gfin.ins, xd_last.ins, sync=False, reason="GP last")
        hooks += [last_act, mm, od, gfin]
        for h in hooks:
            dr = nc.sync.drain()
            add_dep_helper(dr.ins, h.ins, sync=True, reason="pre-drain")


_NC_CACHE = {}


def _get_nc(n, fs):
    key = (n, tuple(fs))
    if key not in _NC_CACHE:
        nc = bass.Bass(
            "TRN2", target_bir_lowering=False, debug=False, num_devices=NCORES
        )
        x = nc.dram_tensor("x", [C, n], _dt.float32, kind="ExternalInput").ap()
        t = nc.dram_tensor("t", [n], _dt.uint8, kind="ExternalInput").ap()
        out = nc.dram_tensor(
            "out", [OUTN], _dt.float32, kind="ExternalOutput").ap()
        with TileContext(nc) as tc:
            _body_v5(tc, x, t, out, n, list(fs))
        _NC_CACHE[key] = nc
    return _NC_CACHE[key]


def _run(input, target, trace=False):
    inp = np.asarray(input, dtype=np.float32)
    tgt = np.asarray(target)
    b_, c_, h_, w_ = inp.shape
    assert c_ == C, (b_, c_, h_, w_)
    hw = h_ * w_
    n = b_ * hw // NCORES
    nc = _get_nc(n, BLOCKS)
    x2 = inp.reshape(b_, C, hw)
    t2 = tgt.reshape(b_, hw)
    in_maps = []
    for core in range(NCORES):
        b, off = divmod(core * n, hw)
        in_maps.append({
            "x": np.ascontiguousarray(x2[b, :, off:off + n]),
            "t": np.ascontiguousarray(t2[b, off:off + n]).astype(
                np.uint8, copy=False),
        })
    res = bass_utils.run_bass_kernel_spmd(
        nc, in_maps, core_ids=list(range(NCORES)), trace=trace
    )
    counts = np.zeros(OUTN, np.float64)
    for r in res.results:
        counts += r["out"].astype(np.float64)
    pred = counts[:C]
    inter = counts[C:]
    label = np.bincount(
        np.asarray(target).reshape(-1).astype(np.int64), minlength=C
    )[:C].astype(np.float64)
    union = pred + label - inter
    iou_mean = (inter / union).mean()
    return np.float32(iou_mean), res


def kernel(input, target):
    return _run(input, target)[0]
